# revision 76
# baseline (speedup 1.0000x reference)
"""GQA causal attention (RoPE) on 8 Trainium2 NeuronCores.

Sharding (tensor-parallel over heads, per the hint):
  core c owns q-heads {2c, 2c+1} and kv-head c//2.
  Each core computes its 2 heads' attention over the full sequence and a
  partial output projection out_c.T = wo[:, 128c:128c+128] @ att_c  (shape
  [1024, 4096]); the final all-reduce over cores is the host-side unshard.

Device-side per core (v18 — cross-chunk software pipeline):
  PE busy (~190us) is the binding resource; ScalarE exp (~140us) saturates
  late chunks. The group loop of chunk n interleaves, at spread group
  slots, the prologue of chunk n+1 (split projections, rope, v^T) early
  and a deferred endgame (denominator broadcast, normalize, wo, store;
  chunk k's endgame runs in chunk k+2) late, so ScalarE and the PE never
  drain at chunk boundaries.

  - All matmuls bf16, fp32 PSUM; scores 2-head row-packed (K=64 row tiles
    run CONCURRENTLY via auto tile_position), h1 emitted first (h0
    additionally waits on the krot-duplicate DMA).
  - Causal masking off the PE: exp runs on raw scores; a DVE bf16
    0/1-triangle multiply zeroes the upper triangle of et in SBUF.
  - RoPE from PSUM in fp32: even/odd perm folded into wq/wk host-side,
    sign-folded sin, partition-block swap via SBUF->SBUF DMA (gpsimd ring
    reserved for these; hwdge swaps measured WORSE in steady state).
  - exp on ScalarE via grouped [128, 2, 512] activations with
    diagonal-trimmed 3D APs; ACT table pre-warmed at t0; chunk-0
    attention priority-pinned ahead of the chunk-1 prologue; PE warmup
    bursts beat the HAM cold clock (warmup2 parked in the av bank).
  - AV with ones-augmented V^T (denominators fall out of the matmul);
    V^T via PE transpose; reciprocal via DVE reciprocal_approx_fast.
  - Last-chunk endgame is q-half sliced and DMA-free: K=1 f32r matmuls
    broadcast the a0/a1 ones-rows (zero-padded selector stationaries,
    2-head accumulate) and an off-diagonal identity matmul lane-shifts
    h1; chunks 0-6 keep the sel-matmul + gpsimd d2 + rb1/attS DMA path.
  - x / trig / out use chunk-contiguous host layouts -> 1 bulk DMA per
    chunk each (x+out on the sync ring, trig/consts on the scalar ring).

  Measured: 230.7us HW exec (v12: 244.7us; first correct: 346-388us),
  rel err 3.7e-3.

  Measured dead ends (do not revisit without new evidence): fp8 anywhere
  on q/k/probs/v (quantization scales with sqrt(K) like the signal ->
  4-6% rms); DVE/Schraudolph exp offload at any dosage (late et delivery
  stalls PE AV); ScalarE evacuation copies (stall the exp stream); merged
  [65,2,512] AV matmul (fp32 PSUM out capped at 512 free elems); matmul
  dst partitions starting at 64 (ISA reject); bulk x0 (delays proj(0));
  rope swaps on sync/scalar hwdge (queue contention, +14us); endgame
  rebalance {6:[4,5],7:[6]}; stage_au priority boost (chunk-2 gain offset
  by losses elsewhere). Never allocate a pool tile that goes unwritten -
  it weakens Tile dep tracking (min-join) and causes data races.
"""
import numpy as np
import ml_dtypes
from contextlib import ExitStack

import concourse.bacc as bacc
import concourse.tile as tile
import concourse.mybir as mybir
from concourse.bass_utils import run_bass_kernel_spmd

DIM = 1024
N_HEADS = 16
N_KV = 4
HD = 64
SEQ = 4096
NCORES = 8

SQ = 512            # query-chunk (free dim of score blocks)
HQ = SQ // 2
SK = 128            # key-chunk (partition dim of score blocks)
NQ = SEQ // SQ      # 8
NR = DIM // 128     # 8 contraction chunks for projections
NJ = SEQ // SK      # 32 key chunks

# chunks whose h1 exp runs on DVE for odd groups (Schraudolph bf16-bitcast).
# Measured: any offload makes late chunks WORSE (DVE already carries
# evacuations + normalize there, and late et delivery stalls the PE) — off.
DVE_EXP_CHUNKS = ()
SCE_COPY_KS = ()      # ScalarE stays a pure exp engine (copies stall the stream)
EXPA = 184.6650125 / 8.0        # (2^7/ln2) * 0.125 score scale
EXPB = 16256.0 - 5.59           # 127*2^7 - C (min-max-rel-err offset)

f32 = mybir.dt.float32
f32r = mybir.dt.float32r
bf16 = mybir.dt.bfloat16
i16 = mybir.dt.int16
FT = mybir.ActivationFunctionType
Alu = mybir.AluOpType

_CACHE = {}


def _emit(nc):
    # chunk-contiguous layouts: x_pre[p, n, r, sq] = x[512n+sq, 128r+p]
    xT = nc.dram_tensor("xT", [128, NQ, NR, SQ], bf16, kind="ExternalInput").ap()
    # trig[p, n, 0, sq] = cos4[p, 512n+sq]; [.., 1, ..] = sin4
    trig_d = nc.dram_tensor("trig", [128, NQ, 2, SQ], bf16, kind="ExternalInput").ap()
    wq_l = nc.dram_tensor("wq_l", [128, DIM], bf16, kind="ExternalInput").ap()
    wkv_l = nc.dram_tensor("wkv_l", [128, DIM], bf16, kind="ExternalInput").ap()
    wo_l = nc.dram_tensor("wo_l", [128, DIM], bf16, kind="ExternalInput").ap()
    tri_d = nc.dram_tensor("tri01", [128, 128], bf16, kind="ExternalInput").ap()
    id_d = nc.dram_tensor("ident", [128, 128], bf16, kind="ExternalInput").ap()
    idr_d = nc.dram_tensor("id64", [HD, 128], f32r, kind="ExternalInput").ap()
    selr_d = nc.dram_tensor("selr", [128, 256], f32r, kind="ExternalInput").ap()
    # ones column of vt comes from a memset, not a DRAM load
    sel_d = nc.dram_tensor("sel2", [2, 128], f32r, kind="ExternalInput").ap()
    # out[p, n, m, sq] = out_partial[128m+p, 512n+sq] (bf16 partials)
    out_d = nc.dram_tensor("out", [128, NQ, NR, SQ], f32, kind="ExternalOutput").ap()

    with tile.TileContext(nc) as tc, ExitStack() as ctx:
        const = ctx.enter_context(tc.tile_pool(name="const", bufs=1))
        main = ctx.enter_context(tc.tile_pool(name="main", bufs=1))

        wq_sb = const.tile([128, DIM], bf16)
        wkv_sb = const.tile([128, DIM], bf16)
        wo_sb = const.tile([128, DIM], bf16)
        tri_sb = const.tile([128, 128], bf16)
        id_sb = const.tile([128, 128], bf16)
        sel_sb = const.tile([2, 128], f32r)
        wrm_sb = const.tile([1, 8], f32)
        selr_sb = const.tile([128, 256], f32r)
        idr_sb = const.tile([HD, 128], f32r)

        qrot = main.tile([128, SEQ], bf16)      # 2 heads d-major (rope'd)
        krot = main.tile([128, SEQ], bf16)      # k duplicated in both halves
        v_sb = main.tile([HD, SEQ], bf16)       # v d-major
        vt = main.tile([128, NJ, 128], bf16)    # v^T + ones column (aligned slots)
        attS = main.tile([128, SEQ], bf16)      # stacked normalized att (j-major)
        att1 = main.tile([HD, SEQ], bf16)       # head-1 att staging (lanes 0-63)

        with (
            tc.tile_pool(name="xp", bufs=2) as xp,       # [128, NR*SQ] x chunks
            tc.tile_pool(name="x0p", bufs=1) as x0p,     # chunk-0 split x
            tc.tile_pool(name="tp", bufs=2) as tp,       # trig chunks
            tc.tile_pool(name="pp", bufs=1, space="PSUM") as pp,
            tc.tile_pool(name="rp", bufs=2) as rp,
            tc.tile_pool(name="sp", bufs=2, space="PSUM") as sp,
            tc.tile_pool(name="ap", bufs=1, space="PSUM") as ap,
            tc.tile_pool(name="ep", bufs=4) as ep,
            tc.tile_pool(name="aup", bufs=3) as aup,     # raw AV staging
            tc.tile_pool(name="rbp", bufs=3) as rbp,
            tc.tile_pool(name="op", bufs=1) as op,       # wo-out staging
        ):
            xsb = {}      # n -> x chunk tile (or list of per-r tiles for n=0)
            trg = {}      # n -> trig chunk tile
            au = {}       # n -> (au0, au1, d2)

            def load_x(n):
                t = xp.tile([128, NR, SQ], bf16, tag="x")
                nc.sync.dma_start(t[:], xT[:, n, :, :])
                xsb[n] = lambda r: t[:, r, :]

            def load_x0():
                ts = []
                for r in range(NR):
                    t = x0p.tile([128, SQ], bf16, tag=f"x0_{r}")
                    eng = nc.sync if r % 2 == 0 else nc.scalar
                    eng.dma_start(t[:], xT[:, 0, r, :])
                    ts.append(t)
                xsb[0] = lambda r: ts[r][:]

            def load_trig(n):
                t = tp.tile([128, 2, SQ], bf16, tag="trig")
                nc.scalar.dma_start(t[:], trig_d[:, n, :, :])
                trg[n] = t

            def proj(n, r0=0, r1=NR):
                # pq/pkv accumulate over contraction chunks [r0, r1)
                if r0 == 0:
                    proj.cur = (pp.tile([128, SQ], f32, tag="pq", name=f"pq_{n}"),
                                pp.tile([128, SQ], f32, tag="pkv", name=f"pkv_{n}"))
                pq, pkv = proj.cur
                xt = xsb[n]
                for r in range(r0, r1):
                    nc.tensor.matmul(pq[:], wq_sb[:, 128 * r:128 * (r + 1)],
                                     xt(r), start=(r == 0), stop=(r == NR - 1))
                    nc.tensor.matmul(pkv[:], wkv_sb[:, 128 * r:128 * (r + 1)],
                                     xt(r), start=(r == 0), stop=(r == NR - 1))
                if r1 == NR:
                    xsb.pop(n)
                return proj.cur

            def rope(n, pq, pkv):
                s0 = n * SQ
                trig = trg.pop(n)
                # chunk 0's swaps are on the startup critical path: use the
                # hwdge fabric (queues are quiet then). Steady-state chunks
                # keep the gpsimd ring — fabric swaps there contend with
                # x/trig/attS traffic (measured +14us).
                e0 = nc.sync if n == 0 else nc.gpsimd
                e1 = nc.scalar if n == 0 else nc.gpsimd
                # ---- q ----
                a_t = rp.tile([128, SQ], f32, tag="ta")
                c_t = rp.tile([128, SQ], f32, tag="tc")
                b_t = rp.tile([128, SQ], f32, tag="tb")
                nc.vector.tensor_mul(a_t[:], pq[:], trig[:, 0, :])
                nc.vector.tensor_mul(c_t[:], pq[:], trig[:, 1, :])
                e0.dma_start(b_t[0:32, :], c_t[32:64, :])
                e1.dma_start(b_t[32:64, :], c_t[0:32, :])
                e0.dma_start(b_t[64:96, :], c_t[96:128, :])
                e1.dma_start(b_t[96:128, :], c_t[64:96, :])
                nc.vector.tensor_add(qrot[:, s0:s0 + SQ], a_t[:], b_t[:])
                # ---- k (rows 64:128; v occupies rows 0:64) ----
                ak = rp.tile([128, SQ], f32, tag="ta")
                ck = rp.tile([128, SQ], f32, tag="tc")
                bk = rp.tile([128, SQ], f32, tag="tb")
                nc.vector.tensor_mul(ak[64:128, :], pkv[64:128, :],
                                     trig[64:128, 0, :])
                nc.vector.tensor_mul(ck[64:128, :], pkv[64:128, :],
                                     trig[64:128, 1, :])
                e0.dma_start(bk[64:96, :], ck[96:128, :])
                e1.dma_start(bk[96:128, :], ck[64:96, :])
                nc.vector.tensor_add(krot[64:128, s0:s0 + SQ], ak[64:128, :], bk[64:128, :])
                # dup on the hwdge fabric: the gpsimd ring is serialized behind
                # the 6 rope swaps (~0.6us each), this was the h0-score gate
                nc.sync.dma_start(krot[0:64, s0:s0 + SQ], krot[64:128, s0:s0 + SQ])
                # ---- v -> bf16 ----
                nc.vector.tensor_copy(v_sb[:, s0:s0 + SQ], pkv[0:64, :])

            def vtrans(n, half=None):
                # v^T via PE transpose (ping-pong pq/pkv banks)
                js = range(4 * n, 4 * n + 4) if half is None else \
                    range(4 * n + 2 * half, 4 * n + 2 * half + 2)
                for i, j in enumerate(js):
                    tg = "pq" if i % 2 == 0 else "pkv"
                    pt = pp.tile([SK, HD], bf16, tag=tg, name=f"pt_{j}")
                    nc.tensor.transpose(pt[:], v_sb[:, SK * j:SK * (j + 1)],
                                        id_sb[0:HD, 0:HD])
                    nc.vector.tensor_copy(vt[:, j, 0:HD], pt[:])

            def stage_au(n):
                # raw AV + denoms out of PSUM; free av banks.
                # last chunk is staged in q-halves so its endgame pipelines.
                a0 = aup.tile([HD + 1, SQ], f32r, tag="au0")
                a1 = aup.tile([HD + 1, SQ], f32r, tag="au1")
                if n < NQ - 1:
                    d2 = rbp.tile([2, SQ], f32r, tag="d2", name=f"d2_{n}")
                else:
                    d2 = None
                sls = ((0, HQ), (HQ, SQ)) if n == NQ - 1 else ((0, SQ),)
                for qa, qb in sls:
                    nc.vector.tensor_copy(a0[:, qa:qb], avs[n][0:HD + 1, 0, qa:qb])
                    nc.vector.tensor_copy(a1[:, qa:qb], avs[n][0:HD + 1, 1, qa:qb])
                    if n < NQ - 1:
                        nc.gpsimd.dma_start(d2[0:1, qa:qb], a0[HD:HD + 1, qa:qb])
                        nc.gpsimd.dma_start(d2[1:2, qa:qb], a1[HD:HD + 1, qa:qb])
                au[n] = (a0, a1, d2)

            def endgame_bc(k, qa=0, qb=SQ):
                # denominator broadcast + fast reciprocal + normalize
                sk0 = k * SQ
                a0, a1, d2 = au[k]
                bc = pp.tile([128, SQ], f32, tag="pq", name=f"bc_{k}_{qa}")
                rb = rbp.tile([128, SQ], f32, tag="rb", name=f"rb_{k}_{qa}")
                if k < NQ - 1:
                    nc.tensor.matmul(bc[:, qa:qb], sel_sb[:], d2[:, qa:qb],
                                     start=True, stop=True)
                    nc.vector.reciprocal_approx_fast(rb[:, qa:qb], bc[:, qa:qb])
                    rb1 = rbp.tile([HD, SQ], f32, tag="rb1", name=f"rb1_{k}_{qa}")
                    nc.sync.dma_start(rb1[:, qa:qb], rb[64:64 + HD, qa:qb])
                    nc.vector.tensor_mul(attS[0:HD, sk0 + qa:sk0 + qb],
                                         a0[0:HD, qa:qb].bitcast(f32),
                                         rb[0:HD, qa:qb])
                    nc.vector.tensor_mul(att1[:, sk0 + qa:sk0 + qb],
                                         a1[0:HD, qa:qb].bitcast(f32),
                                         rb1[:, qa:qb])
                    nc.sync.dma_start(attS[64:128, sk0 + qa:sk0 + qb],
                                      att1[:, sk0 + qa:sk0 + qb])
                else:
                    # DMA-free tail chain: K=1 denominator broadcasts straight
                    # from the a0/a1 ones-rows (zero-padded stationaries,
                    # accumulated) + PE lane-shift for h1 via off-diag identity
                    nc.tensor.matmul(bc[:, qa:qb],
                                     selr_sb[HD:HD + 1, 0:128],
                                     a0[HD:HD + 1, qa:qb],
                                     start=True, stop=False)
                    nc.tensor.matmul(bc[:, qa:qb],
                                     selr_sb[HD:HD + 1, 128:256],
                                     a1[HD:HD + 1, qa:qb],
                                     start=False, stop=True)
                    nc.vector.reciprocal_approx_fast(rb[:, qa:qb], bc[:, qa:qb])
                    sh = pp.tile([128, SQ], f32, tag="pkv", name=f"sh_{qa}")
                    nc.tensor.matmul(sh[:, qa:qb],
                                     idr_sb[:],
                                     a1[0:HD, qa:qb],
                                     start=True, stop=True)
                    nc.vector.tensor_mul(attS[0:HD, sk0 + qa:sk0 + qb],
                                         a0[0:HD, qa:qb].bitcast(f32),
                                         rb[0:HD, qa:qb])
                    nc.vector.tensor_mul(attS[64:128, sk0 + qa:sk0 + qb],
                                         sh[64:128, qa:qb],
                                         rb[64:128, qa:qb])
                if qb == SQ:
                    au.pop(k)

            def endgame_wo(k, half, wide=False, qa=0, qb=SQ):
                # wo matmuls + evacuation; half 0 -> m 0..3, half 1 -> m 4..7
                sk0 = k * SQ
                if half == 0 and qa == 0:
                    endgame_wo.ot = op.tile([128, NR, SQ], f32, tag="ot",
                                            name=f"ot_{k}")
                ot = endgame_wo.ot
                for m in range(4 * half, 4 * half + 4):
                    if wide and m % 2 == 1:
                        pw = sp.tile([128, SQ], f32, tag="sc",
                                     name=f"pw_{k}_{m}_{qa}")
                    else:
                        pw = pp.tile([128, SQ], f32,
                                     tag=("pkv" if m % 2 == 0 else "pq"),
                                     name=f"pw_{k}_{m}_{qa}")
                    nc.tensor.matmul(pw[:, qa:qb], wo_sb[:, 128 * m:128 * (m + 1)],
                                     attS[:, sk0 + qa:sk0 + qb],
                                     start=True, stop=True)
                    if wide and m % 2 == 0:
                        # tail only: the exp stream is done, ScalarE is idle —
                        # split the evacuation drain across both engines
                        nc.scalar.activation(ot[:, m, qa:qb], pw[:, qa:qb],
                                             FT.Copy)
                    else:
                        nc.vector.tensor_copy(ot[:, m, qa:qb], pw[:, qa:qb])
                if wide:
                    # per-2m stores: first pair flies while the second copies
                    nc.sync.dma_start(out_d[:, k, 4 * half:4 * half + 2, qa:qb],
                                      ot[:, 4 * half:4 * half + 2, qa:qb])
                    nc.sync.dma_start(out_d[:, k, 4 * half + 2:4 * half + 4, qa:qb],
                                      ot[:, 4 * half + 2:4 * half + 4, qa:qb])
                else:
                    nc.sync.dma_start(out_d[:, k, 4 * half:4 * half + 4, qa:qb],
                                      ot[:, 4 * half:4 * half + 4, qa:qb])

            avs = {}

            def attention(n, hooks):
                s0 = n * SQ
                nsk = 4 * (n + 1)
                av = ap.tile([HD + 1, 2, SQ], f32, tag="av", name=f"av_{n}")
                avs[n] = av
                pend = []   # (j, et, dd) awaiting AV emission

                def flush_av():
                    j_, et_, dd_ = pend.pop(0)
                    for h_ in (0, 1):
                        nc.tensor.matmul(
                            av[:, h_, dd_:SQ], vt[:, j_, 0:HD + 1],
                            et_[:, h_, dd_:SQ].bitcast(bf16),
                            start=(j_ == 0), stop=(j_ == nsk - 1),
                        )

                def scores(j):
                    delta = SK * j - s0
                    dd = max(0, delta)
                    sc = sp.tile([128, 2, SQ], f32, tag="sc")
                    for h in (1, 0):
                        nc.tensor.matmul(
                            sc[:, h, dd:SQ],
                            krot[64 * h:64 * h + 64, SK * j:SK * (j + 1)],
                            qrot[64 * h:64 * h + 64, s0 + dd:s0 + SQ],
                            start=True, stop=True,
                        )
                    return sc, dd, delta

                def expgrp(j, sc, dd, delta):
                    et = ep.tile([128, 2, SQ], i16, tag="et")
                    if n in DVE_EXP_CHUNKS and j % 2 == 1:
                        # h0 on ScalarE (true exp), h1 on DVE via Schraudolph:
                        # bf16 bitpattern = round(A*s + B), rel err <= ~3%
                        with tc.high_priority(offset=100000):
                            nc.scalar.activation(et[:, 0, dd:].bitcast(bf16),
                                                 sc[:, 0, dd:],
                                                 FT.Exp, scale=0.125)
                        with tc.high_priority(offset=50000):
                            nc.vector.tensor_scalar(et[:, 1, dd:],
                                                    sc[:, 1, dd:],
                                                    EXPA, EXPB,
                                                    Alu.mult, Alu.add)
                    else:
                        with tc.high_priority(offset=100000):
                            nc.scalar.activation(et[:, :, dd:].bitcast(bf16),
                                                 sc[:, :, dd:],
                                                 FT.Exp, scale=0.125)
                    if delta >= 0:
                        # zero the causal upper-triangle on DVE (bf16, SBUF)
                        for h in (0, 1):
                            v = et[:, h, delta:delta + SK].bitcast(bf16)
                            nc.vector.tensor_mul(v, v, tri_sb[:])
                    pend.append((j, et, dd))

                # groups in pairs: both score pairs stay in 64-row tile mode,
                # then exps, then 128-row AV flushes; hooks at every group
                for g0 in range(0, nsk, 2):
                    sa = scores(g0)
                    sb = scores(g0 + 1)
                    expgrp(g0, *sa)
                    if len(pend) > 2:
                        flush_av()
                    hk = hooks.get(g0)
                    if hk is not None:
                        hk()
                    expgrp(g0 + 1, *sb)
                    if len(pend) > 2:
                        flush_av()
                    hk = hooks.get(g0 + 1)
                    if hk is not None:
                        hk()
                while pend:
                    flush_av()
                stage_au(n)

            # ---------------- program ----------------
            # startup loads (weights first; scalar queue stays clean)
            nc.sync.dma_start(wq_sb[:, 0:128], wq_l[:, 0:128])
            nc.sync.dma_start(wq_sb[:, 128:], wq_l[:, 128:])
            nc.sync.dma_start(wkv_sb[:], wkv_l[:])
            load_x0()
            load_trig(0)
            nc.scalar.dma_start(id_sb[:], id_d[:])
            nc.scalar.dma_start(tri_sb[:], tri_d[:])
            nc.gpsimd.memset(vt[:, :, HD:HD + 1], 1.0)
            nc.scalar.dma_start(selr_sb[:], selr_d[:])
            nc.scalar.dma_start(idr_sb[:], idr_d[:])
            nc.scalar.dma_start(sel_sb[:], sel_d[:])
            nc.sync.dma_start(wo_sb[:], wo_l[:])
            # warm the exp ACT table while startup DMAs are in flight
            nc.gpsimd.memset(wrm_sb[:], 0.0)
            nc.scalar.activation(wrm_sb[:], wrm_sb[:], FT.Exp, scale=0.125)

            # chunk 0 prologue (serial)
            load_x(1)
            # PE warmup: dense tiny matmuls on wq while x0 is in flight,
            # so HAM un-throttles before the real projections start
            wu = pp.tile([128, 128], f32, tag="pq", name="warmup")
            for w in range(32):
                nc.tensor.matmul(wu[:], wq_sb[:, 0:128], wq_sb[:, 0:128],
                                 start=True, stop=True)
            pq, pkv = proj(0)
            # warmup2 in the av bank so both sc buffers stay free for chunk 0
            wu2 = ap.tile([128, 128], f32, tag="av", name="warmup2")
            for w in range(14):
                nc.tensor.matmul(wu2[:], wq_sb[:, 0:128], wq_sb[:, 0:128],
                                 start=True, stop=True)
            load_trig(1)
            rope(0, pq, pkv)
            vtrans(0)
            # keep-warm burst: fills the PE gap while rope(0) runs on
            # DVE/DMA, so proj(1) starts at full p-state. The chunk-0
            # attention pin outranks these, so they only occupy idle slots.
            for w in range(32):
                nc.tensor.matmul(wu2[:], wq_sb[:, 0:128], wq_sb[:, 0:128],
                                 start=True, stop=True)
            first_prologue = True

            pending_proj = {}

            for n in range(NQ):
                nsk = 4 * (n + 1)
                hooks = {}
                order = []
                # next-chunk prologue early (its outputs gate the next chunk)
                if n + 1 < NQ:
                    def do_proj(k, r0, r1):
                        got = proj(k, r0, r1)
                        if r1 == NR:
                            pending_proj[k] = got
                    def do_rope(k=n + 1):
                        pq_, pkv_ = pending_proj.pop(k)
                        rope(k, pq_, pkv_)
                    order += [lambda k=n + 1: load_x(k + 1) if k + 1 < NQ else None,
                              lambda k=n + 1: do_proj(k, 0, 2),
                              lambda k=n + 1: do_proj(k, 2, 4),
                              lambda k=n + 1: do_proj(k, 4, 6),
                              lambda k=n + 1: do_proj(k, 6, 8),
                              do_rope,
                              lambda k=n + 1: (load_trig(k + 1)
                                               if k + 1 < NQ else None),
                              lambda k=n + 1: vtrans(k, 0),
                              lambda k=n + 1: vtrans(k, 1)]
                # deferred endgames (chunk k+2 for k<=4; chunk 7 gets 5+6)
                for k in {2: [0], 3: [1], 4: [2], 5: [3], 6: [4],
                          7: [5, 6]}.get(n, []):
                    order += [lambda k=k: endgame_bc(k),
                              lambda k=k: endgame_wo(k, 0),
                              lambda k=k: endgame_wo(k, 1)]
                if n == 0:
                    # pin chunk-0 attention ahead of the chunk-1 prologue in
                    # the scheduler's priority heap (it otherwise runs
                    # proj(1)/rope(1) first, delaying the first exps ~8us)
                    with tc.high_priority(offset=200000):
                        attention(n, {})
                    for fn in order:
                        fn()
                else:
                    # spread across groups (stacking), first hook at gi=1
                    for i, fn in enumerate(order):
                        gi = 1 + (i * (nsk - 1)) // len(order)
                        prev = hooks.get(gi)
                        hooks[gi] = (fn if prev is None else
                                     (lambda a=prev, b=fn: (a(), b())))
                    attention(n, hooks)

            # last chunk's endgame pipelined in q-halves
            for qa, qb in ((0, HQ), (HQ, SQ)):
                endgame_bc(NQ - 1, qa, qb)
                endgame_wo(NQ - 1, 0, wide=True, qa=qa, qb=qb)
                endgame_wo(NQ - 1, 1, wide=True, qa=qa, qb=qb)


def _build():
    if "nc" in _CACHE:
        return _CACHE["nc"]
    nc = bacc.Bacc("TRN2", target_bir_lowering=False, debug=False, num_devices=NCORES)
    _emit(nc)
    nc.compile()
    _CACHE["nc"] = nc
    return nc


def _host_inputs(x, freqs_cos, freqs_sin, wq, wk, wv, wo):
    x = np.asarray(x, np.float32)
    freqs_cos = np.asarray(freqs_cos, np.float32)
    freqs_sin = np.asarray(freqs_sin, np.float32)
    wq = np.asarray(wq, np.float32)
    wk = np.asarray(wk, np.float32)
    wv = np.asarray(wv, np.float32)
    wo = np.asarray(wo, np.float32)

    # x_pre[p, n, r, sq] = x[512n+sq, 128r+p]
    xv = x[0].reshape(NQ, SQ, NR, 128)
    x_pre = np.ascontiguousarray(xv.transpose(3, 0, 2, 1)).astype(ml_dtypes.bfloat16)

    cosT = freqs_cos.T                                              # [32, 4096]
    sinT = freqs_sin.T
    cos4 = np.tile(cosT, (4, 1)).reshape(128, NQ, SQ)
    sin4 = np.concatenate([sinT, -sinT, sinT, -sinT], axis=0).reshape(128, NQ, SQ)
    trig = np.ascontiguousarray(np.stack([cos4, sin4], axis=2)).astype(
        ml_dtypes.bfloat16)                                         # [128, 8, 2, 512]

    # 0/1 lower-triangle keep-mask for the diagonal SK x SK block
    p = np.arange(SK)[:, None]
    f = np.arange(SK)[None, :]
    tri01 = (p <= f).astype(ml_dtypes.bfloat16)                     # [128, 128]

    ident = np.eye(128, dtype=ml_dtypes.bfloat16)
    sel2 = np.zeros((2, 128), dtype=np.float32)
    sel2[0, 0:64] = 1.0
    sel2[1, 64:128] = 1.0
    # tail constants: K=1 denominator-broadcast stationaries (row 64) and the
    # h1 lane-shift identity (rows k -> out partition k+64)
    selr = np.zeros((128, 256), dtype=np.float32)
    selr[:, 0:64] = 1.0
    selr[:, 192:256] = 1.0
    id64 = np.zeros((HD, 128), dtype=np.float32)
    id64[np.arange(HD), np.arange(HD) + 64] = 1.0

    perm = np.concatenate([np.arange(0, HD, 2), np.arange(1, HD, 2)])

    def fold(w):  # [128(m), 1024(d)] -> lhsT layout [128(p), 8r*128+m]
        return np.ascontiguousarray(
            w.reshape(128, NR, 128).transpose(2, 1, 0).reshape(128, DIM)
        ).astype(ml_dtypes.bfloat16)

    in_maps = []
    for c in range(NCORES):
        g = c // 2
        wq_c = wq[128 * c:128 * (c + 1)].reshape(2, HD, DIM)[:, perm, :].reshape(128, DIM)
        wk_g = wk[HD * g:HD * (g + 1)][perm]
        wv_g = wv[HD * g:HD * (g + 1)]
        wkv_c = np.concatenate([wv_g, wk_g], axis=0)        # v rows 0:64, k rows 64:128
        wo_c = np.ascontiguousarray(wo[:, 128 * c:128 * (c + 1)].T).astype(
            ml_dtypes.bfloat16)                              # [128(j), 1024(o)]
        in_maps.append({
            "xT": x_pre,
            "wq_l": fold(wq_c),
            "wkv_l": fold(wkv_c),
            "wo_l": wo_c,
            "trig": trig,
            "tri01": tri01,
            "ident": ident,
            "id64": id64,
            "selr": selr,
            "sel2": sel2,
        })
    return in_maps


def kernel(x, freqs_cos, freqs_sin, wq, wk, wv, wo, _trace=False, _trace_kwargs=None):
    nc = _build()
    in_maps = _host_inputs(x, freqs_cos, freqs_sin, wq, wk, wv, wo)
    kw = {}
    if _trace:
        kw.update(trace=True, **(_trace_kwargs or {}))
    res = run_bass_kernel_spmd(nc, in_maps, core_ids=list(range(NCORES)), **kw)
    acc = np.zeros((128, NQ, NR, SQ), np.float32)
    for c in range(NCORES):
        acc += np.asarray(res.results[c]["out"], np.float32)
    # out[p, n, m, sq] -> [512n+sq, 128m+p]
    out = np.ascontiguousarray(acc.transpose(1, 3, 2, 0)).reshape(1, SEQ, DIM)
    if _trace:
        kernel._last_results = res
    return out



# revision 77
# speedup vs baseline: 1.0219x; 1.0219x over previous
"""GQA causal attention (RoPE) on 8 Trainium2 NeuronCores.

Sharding (tensor-parallel over heads, per the hint):
  core c owns q-heads {2c, 2c+1} and kv-head c//2.
  Each core computes its 2 heads' attention over the full sequence and a
  partial output projection out_c.T = wo[:, 128c:128c+128] @ att_c  (shape
  [1024, 4096]); the final all-reduce over cores is the host-side unshard.

Device-side per core (v18 — cross-chunk software pipeline):
  PE busy (~190us) is the binding resource; ScalarE exp (~140us) saturates
  late chunks. The group loop of chunk n interleaves, at spread group
  slots, the prologue of chunk n+1 (split projections, rope, v^T) early
  and a deferred endgame (denominator broadcast, normalize, wo, store;
  chunk k's endgame runs in chunk k+2) late, so ScalarE and the PE never
  drain at chunk boundaries.

  - All matmuls bf16, fp32 PSUM; scores 2-head row-packed (K=64 row tiles
    run CONCURRENTLY via auto tile_position), h1 emitted first (h0
    additionally waits on the krot-duplicate DMA).
  - Causal masking off the PE: exp runs on raw scores; a DVE bf16
    0/1-triangle multiply zeroes the upper triangle of et in SBUF.
  - RoPE from PSUM in fp32: even/odd perm folded into wq/wk host-side,
    sign-folded sin, partition-block swap via SBUF->SBUF DMA (gpsimd ring
    reserved for these; hwdge swaps measured WORSE in steady state).
  - exp on ScalarE via grouped [128, 2, 512] activations with
    diagonal-trimmed 3D APs; ACT table pre-warmed at t0; chunk-0
    attention priority-pinned ahead of the chunk-1 prologue; PE warmup
    bursts beat the HAM cold clock (warmup2 parked in the av bank).
  - AV with ones-augmented V^T (denominators fall out of the matmul);
    V^T via PE transpose; reciprocal via DVE reciprocal_approx_fast.
  - Last-chunk endgame is q-half sliced and DMA-free: K=1 f32r matmuls
    broadcast the a0/a1 ones-rows (zero-padded selector stationaries,
    2-head accumulate) and an off-diagonal identity matmul lane-shifts
    h1; chunks 0-6 keep the sel-matmul + gpsimd d2 + rb1/attS DMA path.
  - x / trig / out use chunk-contiguous host layouts -> 1 bulk DMA per
    chunk each (x+out on the sync ring, trig/consts on the scalar ring).

  Measured: 230.7us HW exec (v12: 244.7us; first correct: 346-388us),
  rel err 3.7e-3.

  Measured dead ends (do not revisit without new evidence): fp8 anywhere
  on q/k/probs/v (quantization scales with sqrt(K) like the signal ->
  4-6% rms); DVE/Schraudolph exp offload at any dosage (late et delivery
  stalls PE AV); ScalarE evacuation copies (stall the exp stream); merged
  [65,2,512] AV matmul (fp32 PSUM out capped at 512 free elems); matmul
  dst partitions starting at 64 (ISA reject); bulk x0 (delays proj(0));
  rope swaps on sync/scalar hwdge (queue contention, +14us); endgame
  rebalance {6:[4,5],7:[6]}; stage_au priority boost (chunk-2 gain offset
  by losses elsewhere). Never allocate a pool tile that goes unwritten -
  it weakens Tile dep tracking (min-join) and causes data races.
"""
import numpy as np
import ml_dtypes
from contextlib import ExitStack

import concourse.bacc as bacc
import concourse.tile as tile
import concourse.mybir as mybir
from concourse.bass_utils import run_bass_kernel_spmd

DIM = 1024
N_HEADS = 16
N_KV = 4
HD = 64
SEQ = 4096
NCORES = 8

SQ = 512            # query-chunk (free dim of score blocks)
HQ = SQ // 2
SK = 128            # key-chunk (partition dim of score blocks)
NQ = SEQ // SQ      # 8
NR = DIM // 128     # 8 contraction chunks for projections
NJ = SEQ // SK      # 32 key chunks

# chunks whose h1 exp runs on DVE for odd groups (Schraudolph bf16-bitcast).
# Measured: any offload makes late chunks WORSE (DVE already carries
# evacuations + normalize there, and late et delivery stalls the PE) — off.
DVE_EXP_CHUNKS = ()
SCE_COPY_KS = ()      # ScalarE stays a pure exp engine (copies stall the stream)
EXPA = 184.6650125 / 8.0        # (2^7/ln2) * 0.125 score scale
EXPB = 16256.0 - 5.59           # 127*2^7 - C (min-max-rel-err offset)

f32 = mybir.dt.float32
f32r = mybir.dt.float32r
bf16 = mybir.dt.bfloat16
i16 = mybir.dt.int16
FT = mybir.ActivationFunctionType
Alu = mybir.AluOpType

_CACHE = {}


def _emit(nc):
    # chunk-contiguous layouts: x_pre[p, n, r, sq] = x[512n+sq, 128r+p]
    xT = nc.dram_tensor("xT", [128, NQ, NR, SQ], bf16, kind="ExternalInput").ap()
    # trig[p, n, 0, sq] = cos4[p, 512n+sq]; [.., 1, ..] = sin4
    trig_d = nc.dram_tensor("trig", [128, NQ, 2, SQ], bf16, kind="ExternalInput").ap()
    wq_l = nc.dram_tensor("wq_l", [128, DIM], bf16, kind="ExternalInput").ap()
    wkv_l = nc.dram_tensor("wkv_l", [128, DIM], bf16, kind="ExternalInput").ap()
    wo_l = nc.dram_tensor("wo_l", [128, DIM], bf16, kind="ExternalInput").ap()
    tri_d = nc.dram_tensor("tri01", [128, 128], bf16, kind="ExternalInput").ap()
    id_d = nc.dram_tensor("ident", [128, 128], bf16, kind="ExternalInput").ap()
    idr_d = nc.dram_tensor("id64", [HD, 128], f32r, kind="ExternalInput").ap()
    selr_d = nc.dram_tensor("selr", [128, 256], f32r, kind="ExternalInput").ap()
    # ones column of vt comes from a memset, not a DRAM load
    sel_d = nc.dram_tensor("sel2", [2, 128], f32r, kind="ExternalInput").ap()
    # out[p, n, m, sq] = out_partial[128m+p, 512n+sq] (bf16 partials)
    out_d = nc.dram_tensor("out", [128, NQ, NR, SQ], f32, kind="ExternalOutput").ap()

    with tile.TileContext(nc) as tc, ExitStack() as ctx:
        const = ctx.enter_context(tc.tile_pool(name="const", bufs=1))
        main = ctx.enter_context(tc.tile_pool(name="main", bufs=1))

        wq_sb = const.tile([128, DIM], bf16)
        wkv_sb = const.tile([128, DIM], bf16)
        wo_sb = const.tile([128, DIM], bf16)
        tri_sb = const.tile([128, 128], bf16)
        id_sb = const.tile([128, 128], bf16)
        sel_sb = const.tile([2, 128], f32r)
        wrm_sb = const.tile([1, 8], f32)
        selr_sb = const.tile([128, 256], f32r)
        idr_sb = const.tile([HD, 128], f32r)

        qrot = main.tile([128, SEQ], bf16)      # 2 heads d-major (rope'd)
        krot = main.tile([128, SEQ], bf16)      # k duplicated in both halves
        v_sb = main.tile([HD, SEQ], bf16)       # v d-major
        vt = main.tile([128, NJ, 128], bf16)    # v^T + ones column (aligned slots)
        attS = main.tile([128, SEQ], bf16)      # stacked normalized att (j-major)
        att1 = main.tile([HD, SEQ], bf16)       # head-1 att staging (lanes 0-63)

        with (
            tc.tile_pool(name="xp", bufs=2) as xp,       # [128, NR*SQ] x chunks
            tc.tile_pool(name="x0p", bufs=1) as x0p,     # chunk-0 split x
            tc.tile_pool(name="tp", bufs=2) as tp,       # trig chunks
            tc.tile_pool(name="pp", bufs=1, space="PSUM") as pp,
            tc.tile_pool(name="rp", bufs=2) as rp,
            tc.tile_pool(name="sp", bufs=2, space="PSUM") as sp,
            tc.tile_pool(name="ap", bufs=1, space="PSUM") as ap,
            tc.tile_pool(name="ep", bufs=4) as ep,
            tc.tile_pool(name="aup", bufs=3) as aup,     # raw AV staging
            tc.tile_pool(name="rbp", bufs=3) as rbp,
            tc.tile_pool(name="op", bufs=1) as op,       # wo-out staging
        ):
            xsb = {}      # n -> x chunk tile (or list of per-r tiles for n=0)
            trg = {}      # n -> trig chunk tile
            au = {}       # n -> (au0, au1, d2)

            def load_x(n):
                t = xp.tile([128, NR, SQ], bf16, tag="x")
                nc.sync.dma_start(t[:], xT[:, n, :, :])
                xsb[n] = lambda r: t[:, r, :]

            def load_x0():
                ts = []
                for r in range(NR):
                    t = x0p.tile([128, SQ], bf16, tag=f"x0_{r}")
                    eng = nc.sync if r % 2 == 0 else nc.scalar
                    eng.dma_start(t[:], xT[:, 0, r, :])
                    ts.append(t)
                xsb[0] = lambda r: ts[r][:]

            def load_trig(n):
                t = tp.tile([128, 2, SQ], bf16, tag="trig")
                nc.scalar.dma_start(t[:], trig_d[:, n, :, :])
                trg[n] = t

            def proj(n, r0=0, r1=NR):
                # pq/pkv accumulate over contraction chunks [r0, r1)
                if r0 == 0:
                    proj.cur = (pp.tile([128, SQ], f32, tag="pq", name=f"pq_{n}"),
                                pp.tile([128, SQ], f32, tag="pkv", name=f"pkv_{n}"))
                pq, pkv = proj.cur
                xt = xsb[n]
                for r in range(r0, r1):
                    nc.tensor.matmul(pq[:], wq_sb[:, 128 * r:128 * (r + 1)],
                                     xt(r), start=(r == 0), stop=(r == NR - 1))
                    nc.tensor.matmul(pkv[:], wkv_sb[:, 128 * r:128 * (r + 1)],
                                     xt(r), start=(r == 0), stop=(r == NR - 1))
                if r1 == NR:
                    xsb.pop(n)
                return proj.cur

            def rope(n, pq, pkv):
                s0 = n * SQ
                trig = trg.pop(n)
                # chunk 0's swaps are on the startup critical path: use the
                # hwdge fabric (queues are quiet then). Steady-state chunks
                # keep the gpsimd ring — fabric swaps there contend with
                # x/trig/attS traffic (measured +14us).
                e0 = nc.sync if n == 0 else nc.gpsimd
                e1 = nc.scalar if n == 0 else nc.gpsimd
                # ---- q ----
                a_t = rp.tile([128, SQ], f32, tag="ta")
                c_t = rp.tile([128, SQ], f32, tag="tc")
                b_t = rp.tile([128, SQ], f32, tag="tb")
                nc.vector.tensor_mul(a_t[:], pq[:], trig[:, 0, :])
                nc.vector.tensor_mul(c_t[:], pq[:], trig[:, 1, :])
                e0.dma_start(b_t[0:32, :], c_t[32:64, :])
                e1.dma_start(b_t[32:64, :], c_t[0:32, :])
                e0.dma_start(b_t[64:96, :], c_t[96:128, :])
                e1.dma_start(b_t[96:128, :], c_t[64:96, :])
                nc.vector.tensor_add(qrot[:, s0:s0 + SQ], a_t[:], b_t[:])
                # ---- k (rows 64:128; v occupies rows 0:64) ----
                ak = rp.tile([128, SQ], f32, tag="ta")
                ck = rp.tile([128, SQ], f32, tag="tc")
                bk = rp.tile([128, SQ], f32, tag="tb")
                nc.vector.tensor_mul(ak[64:128, :], pkv[64:128, :],
                                     trig[64:128, 0, :])
                nc.vector.tensor_mul(ck[64:128, :], pkv[64:128, :],
                                     trig[64:128, 1, :])
                e0.dma_start(bk[64:96, :], ck[96:128, :])
                e1.dma_start(bk[96:128, :], ck[64:96, :])
                nc.vector.tensor_add(krot[64:128, s0:s0 + SQ], ak[64:128, :], bk[64:128, :])
                # dup on the hwdge fabric: the gpsimd ring is serialized behind
                # the 6 rope swaps (~0.6us each), this was the h0-score gate
                nc.sync.dma_start(krot[0:64, s0:s0 + SQ], krot[64:128, s0:s0 + SQ])
                # ---- v -> bf16 ----
                nc.vector.tensor_copy(v_sb[:, s0:s0 + SQ], pkv[0:64, :])

            def vtrans(n, half=None):
                # v^T via PE transpose (ping-pong pq/pkv banks)
                js = range(4 * n, 4 * n + 4) if half is None else \
                    range(4 * n + 2 * half, 4 * n + 2 * half + 2)
                for i, j in enumerate(js):
                    tg = "pq" if i % 2 == 0 else "pkv"
                    pt = pp.tile([SK, HD], bf16, tag=tg, name=f"pt_{j}")
                    nc.tensor.transpose(pt[:], v_sb[:, SK * j:SK * (j + 1)],
                                        id_sb[0:HD, 0:HD])
                    nc.vector.tensor_copy(vt[:, j, 0:HD], pt[:])

            def stage_au(n):
                # raw AV + denoms out of PSUM; free av banks.
                # last chunk is staged in q-halves so its endgame pipelines.
                a0 = aup.tile([HD + 1, SQ], f32r, tag="au0")
                a1 = aup.tile([HD + 1, SQ], f32r, tag="au1")
                if n < NQ - 1:
                    d2 = rbp.tile([2, SQ], f32r, tag="d2", name=f"d2_{n}")
                else:
                    d2 = None
                sls = ((0, HQ), (HQ, SQ)) if n == NQ - 1 else ((0, SQ),)
                for qa, qb in sls:
                    nc.vector.tensor_copy(a0[:, qa:qb], avs[n][0:HD + 1, 0, qa:qb])
                    nc.vector.tensor_copy(a1[:, qa:qb], avs[n][0:HD + 1, 1, qa:qb])
                    if n < NQ - 1:
                        nc.gpsimd.dma_start(d2[0:1, qa:qb], a0[HD:HD + 1, qa:qb])
                        nc.gpsimd.dma_start(d2[1:2, qa:qb], a1[HD:HD + 1, qa:qb])
                au[n] = (a0, a1, d2)

            def endgame_bc(k, qa=0, qb=SQ):
                # denominator broadcast + fast reciprocal + normalize
                sk0 = k * SQ
                a0, a1, d2 = au[k]
                bc = pp.tile([128, SQ], f32, tag="pq", name=f"bc_{k}_{qa}")
                rb = rbp.tile([128, SQ], f32, tag="rb", name=f"rb_{k}_{qa}")
                if k < NQ - 1:
                    nc.tensor.matmul(bc[:, qa:qb], sel_sb[:], d2[:, qa:qb],
                                     start=True, stop=True)
                    nc.vector.reciprocal_approx_fast(rb[:, qa:qb], bc[:, qa:qb])
                    rb1 = rbp.tile([HD, SQ], f32, tag="rb1", name=f"rb1_{k}_{qa}")
                    nc.sync.dma_start(rb1[:, qa:qb], rb[64:64 + HD, qa:qb])
                    nc.vector.tensor_mul(attS[0:HD, sk0 + qa:sk0 + qb],
                                         a0[0:HD, qa:qb].bitcast(f32),
                                         rb[0:HD, qa:qb])
                    nc.vector.tensor_mul(att1[:, sk0 + qa:sk0 + qb],
                                         a1[0:HD, qa:qb].bitcast(f32),
                                         rb1[:, qa:qb])
                    nc.sync.dma_start(attS[64:128, sk0 + qa:sk0 + qb],
                                      att1[:, sk0 + qa:sk0 + qb])
                else:
                    # DMA-free tail chain: K=1 denominator broadcasts straight
                    # from the a0/a1 ones-rows (zero-padded stationaries,
                    # accumulated) + PE lane-shift for h1 via off-diag identity
                    nc.tensor.matmul(bc[:, qa:qb],
                                     selr_sb[HD:HD + 1, 0:128],
                                     a0[HD:HD + 1, qa:qb],
                                     start=True, stop=False)
                    nc.tensor.matmul(bc[:, qa:qb],
                                     selr_sb[HD:HD + 1, 128:256],
                                     a1[HD:HD + 1, qa:qb],
                                     start=False, stop=True)
                    nc.vector.reciprocal_approx_fast(rb[:, qa:qb], bc[:, qa:qb])
                    sh = pp.tile([128, SQ], f32, tag="pkv", name=f"sh_{qa}")
                    nc.tensor.matmul(sh[:, qa:qb],
                                     idr_sb[:],
                                     a1[0:HD, qa:qb],
                                     start=True, stop=True)
                    nc.vector.tensor_mul(attS[0:HD, sk0 + qa:sk0 + qb],
                                         a0[0:HD, qa:qb].bitcast(f32),
                                         rb[0:HD, qa:qb])
                    nc.vector.tensor_mul(attS[64:128, sk0 + qa:sk0 + qb],
                                         sh[64:128, qa:qb],
                                         rb[64:128, qa:qb])
                if qb == SQ:
                    au.pop(k)

            def endgame_wo(k, half, wide=False, qa=0, qb=SQ):
                # wo matmuls + evacuation; half 0 -> m 0..3, half 1 -> m 4..7
                sk0 = k * SQ
                if half == 0 and qa == 0:
                    endgame_wo.ot = op.tile([128, NR, SQ], f32, tag="ot",
                                            name=f"ot_{k}")
                ot = endgame_wo.ot
                for m in range(4 * half, 4 * half + 4):
                    if wide and m % 2 == 1:
                        pw = sp.tile([128, SQ], f32, tag="sc",
                                     name=f"pw_{k}_{m}_{qa}")
                    else:
                        pw = pp.tile([128, SQ], f32,
                                     tag=("pkv" if m % 2 == 0 else "pq"),
                                     name=f"pw_{k}_{m}_{qa}")
                    nc.tensor.matmul(pw[:, qa:qb], wo_sb[:, 128 * m:128 * (m + 1)],
                                     attS[:, sk0 + qa:sk0 + qb],
                                     start=True, stop=True)
                    if k in SCE_COPY_KS and m % 2 == 0:
                        nc.scalar.activation(ot[:, m, qa:qb], pw[:, qa:qb],
                                             FT.Copy)
                    else:
                        nc.vector.tensor_copy(ot[:, m, qa:qb], pw[:, qa:qb])
                nc.sync.dma_start(out_d[:, k, 4 * half:4 * half + 4, qa:qb],
                                  ot[:, 4 * half:4 * half + 4, qa:qb])

            avs = {}

            def attention(n, hooks):
                s0 = n * SQ
                nsk = 4 * (n + 1)
                av = ap.tile([HD + 1, 2, SQ], f32, tag="av", name=f"av_{n}")
                avs[n] = av
                pend = []   # (j, et, dd) awaiting AV emission

                def flush_av():
                    j_, et_, dd_ = pend.pop(0)
                    for h_ in (0, 1):
                        nc.tensor.matmul(
                            av[:, h_, dd_:SQ], vt[:, j_, 0:HD + 1],
                            et_[:, h_, dd_:SQ].bitcast(bf16),
                            start=(j_ == 0), stop=(j_ == nsk - 1),
                        )

                def scores(j):
                    delta = SK * j - s0
                    dd = max(0, delta)
                    sc = sp.tile([128, 2, SQ], f32, tag="sc")
                    for h in (1, 0):
                        nc.tensor.matmul(
                            sc[:, h, dd:SQ],
                            krot[64 * h:64 * h + 64, SK * j:SK * (j + 1)],
                            qrot[64 * h:64 * h + 64, s0 + dd:s0 + SQ],
                            start=True, stop=True,
                        )
                    return sc, dd, delta

                def expgrp(j, sc, dd, delta):
                    et = ep.tile([128, 2, SQ], i16, tag="et")
                    if n in DVE_EXP_CHUNKS and j % 2 == 1:
                        # h0 on ScalarE (true exp), h1 on DVE via Schraudolph:
                        # bf16 bitpattern = round(A*s + B), rel err <= ~3%
                        with tc.high_priority(offset=100000):
                            nc.scalar.activation(et[:, 0, dd:].bitcast(bf16),
                                                 sc[:, 0, dd:],
                                                 FT.Exp, scale=0.125)
                        with tc.high_priority(offset=50000):
                            nc.vector.tensor_scalar(et[:, 1, dd:],
                                                    sc[:, 1, dd:],
                                                    EXPA, EXPB,
                                                    Alu.mult, Alu.add)
                    else:
                        with tc.high_priority(offset=100000):
                            nc.scalar.activation(et[:, :, dd:].bitcast(bf16),
                                                 sc[:, :, dd:],
                                                 FT.Exp, scale=0.125)
                    if delta >= 0:
                        # zero the causal upper-triangle on DVE (bf16, SBUF)
                        for h in (0, 1):
                            v = et[:, h, delta:delta + SK].bitcast(bf16)
                            nc.vector.tensor_mul(v, v, tri_sb[:])
                    pend.append((j, et, dd))

                # groups in pairs: both score pairs stay in 64-row tile mode,
                # then exps, then 128-row AV flushes; hooks at every group
                for g0 in range(0, nsk, 2):
                    sa = scores(g0)
                    sb = scores(g0 + 1)
                    expgrp(g0, *sa)
                    if len(pend) > 2:
                        flush_av()
                    hk = hooks.get(g0)
                    if hk is not None:
                        hk()
                    expgrp(g0 + 1, *sb)
                    if len(pend) > 2:
                        flush_av()
                    hk = hooks.get(g0 + 1)
                    if hk is not None:
                        hk()
                while pend:
                    flush_av()
                stage_au(n)

            # ---------------- program ----------------
            # startup loads (weights first; scalar queue stays clean)
            nc.sync.dma_start(wq_sb[:, 0:128], wq_l[:, 0:128])
            nc.sync.dma_start(wq_sb[:, 128:], wq_l[:, 128:])
            nc.sync.dma_start(wkv_sb[:], wkv_l[:])
            load_x0()
            load_trig(0)
            nc.scalar.dma_start(id_sb[:], id_d[:])
            nc.scalar.dma_start(tri_sb[:], tri_d[:])
            nc.gpsimd.memset(vt[:, :, HD:HD + 1], 1.0)
            nc.scalar.dma_start(selr_sb[:], selr_d[:])
            nc.scalar.dma_start(idr_sb[:], idr_d[:])
            nc.scalar.dma_start(sel_sb[:], sel_d[:])
            nc.sync.dma_start(wo_sb[:], wo_l[:])
            # warm the exp ACT table while startup DMAs are in flight
            nc.gpsimd.memset(wrm_sb[:], 0.0)
            nc.scalar.activation(wrm_sb[:], wrm_sb[:], FT.Exp, scale=0.125)

            # chunk 0 prologue (serial)
            load_x(1)
            # PE warmup: dense tiny matmuls on wq while x0 is in flight,
            # so HAM un-throttles before the real projections start
            wu = pp.tile([128, 128], f32, tag="pq", name="warmup")
            for w in range(32):
                nc.tensor.matmul(wu[:], wq_sb[:, 0:128], wq_sb[:, 0:128],
                                 start=True, stop=True)
            pq, pkv = proj(0)
            # warmup2 in the av bank so both sc buffers stay free for chunk 0
            wu2 = ap.tile([128, 128], f32, tag="av", name="warmup2")
            for w in range(14):
                nc.tensor.matmul(wu2[:], wq_sb[:, 0:128], wq_sb[:, 0:128],
                                 start=True, stop=True)
            load_trig(1)
            rope(0, pq, pkv)
            vtrans(0)
            # keep-warm burst: fills the PE gap while rope(0) runs on
            # DVE/DMA, so proj(1) starts at full p-state. The chunk-0
            # attention pin outranks these, so they only occupy idle slots.
            for w in range(32):
                nc.tensor.matmul(wu2[:], wq_sb[:, 0:128], wq_sb[:, 0:128],
                                 start=True, stop=True)
            first_prologue = True

            pending_proj = {}

            for n in range(NQ):
                nsk = 4 * (n + 1)
                hooks = {}
                order = []
                # next-chunk prologue early (its outputs gate the next chunk)
                if n + 1 < NQ:
                    def do_proj(k, r0, r1):
                        got = proj(k, r0, r1)
                        if r1 == NR:
                            pending_proj[k] = got
                    def do_rope(k=n + 1):
                        pq_, pkv_ = pending_proj.pop(k)
                        rope(k, pq_, pkv_)
                    order += [lambda k=n + 1: load_x(k + 1) if k + 1 < NQ else None,
                              lambda k=n + 1: do_proj(k, 0, 2),
                              lambda k=n + 1: do_proj(k, 2, 4),
                              lambda k=n + 1: do_proj(k, 4, 6),
                              lambda k=n + 1: do_proj(k, 6, 8),
                              do_rope,
                              lambda k=n + 1: (load_trig(k + 1)
                                               if k + 1 < NQ else None),
                              lambda k=n + 1: vtrans(k, 0),
                              lambda k=n + 1: vtrans(k, 1)]
                # deferred endgames (chunk k+2 for k<=4; chunk 7 gets 5+6)
                for k in {2: [0], 3: [1], 4: [2], 5: [3], 6: [4],
                          7: [5, 6]}.get(n, []):
                    order += [lambda k=k: endgame_bc(k),
                              lambda k=k: endgame_wo(k, 0),
                              lambda k=k: endgame_wo(k, 1)]
                if n == 0:
                    # pin chunk-0 attention ahead of the chunk-1 prologue in
                    # the scheduler's priority heap (it otherwise runs
                    # proj(1)/rope(1) first, delaying the first exps ~8us)
                    with tc.high_priority(offset=200000):
                        attention(n, {})
                    for fn in order:
                        fn()
                else:
                    # spread across groups (stacking), first hook at gi=1
                    for i, fn in enumerate(order):
                        gi = 1 + (i * (nsk - 1)) // len(order)
                        prev = hooks.get(gi)
                        hooks[gi] = (fn if prev is None else
                                     (lambda a=prev, b=fn: (a(), b())))
                    attention(n, hooks)

            # last chunk's endgame pipelined in q-halves
            for qa, qb in ((0, HQ), (HQ, SQ)):
                endgame_bc(NQ - 1, qa, qb)
                endgame_wo(NQ - 1, 0, wide=True, qa=qa, qb=qb)
                endgame_wo(NQ - 1, 1, wide=True, qa=qa, qb=qb)


def _build():
    if "nc" in _CACHE:
        return _CACHE["nc"]
    nc = bacc.Bacc("TRN2", target_bir_lowering=False, debug=False, num_devices=NCORES)
    _emit(nc)
    nc.compile()
    _CACHE["nc"] = nc
    return nc


def _host_inputs(x, freqs_cos, freqs_sin, wq, wk, wv, wo):
    x = np.asarray(x, np.float32)
    freqs_cos = np.asarray(freqs_cos, np.float32)
    freqs_sin = np.asarray(freqs_sin, np.float32)
    wq = np.asarray(wq, np.float32)
    wk = np.asarray(wk, np.float32)
    wv = np.asarray(wv, np.float32)
    wo = np.asarray(wo, np.float32)

    # x_pre[p, n, r, sq] = x[512n+sq, 128r+p]
    xv = x[0].reshape(NQ, SQ, NR, 128)
    x_pre = np.ascontiguousarray(xv.transpose(3, 0, 2, 1)).astype(ml_dtypes.bfloat16)

    cosT = freqs_cos.T                                              # [32, 4096]
    sinT = freqs_sin.T
    cos4 = np.tile(cosT, (4, 1)).reshape(128, NQ, SQ)
    sin4 = np.concatenate([sinT, -sinT, sinT, -sinT], axis=0).reshape(128, NQ, SQ)
    trig = np.ascontiguousarray(np.stack([cos4, sin4], axis=2)).astype(
        ml_dtypes.bfloat16)                                         # [128, 8, 2, 512]

    # 0/1 lower-triangle keep-mask for the diagonal SK x SK block
    p = np.arange(SK)[:, None]
    f = np.arange(SK)[None, :]
    tri01 = (p <= f).astype(ml_dtypes.bfloat16)                     # [128, 128]

    ident = np.eye(128, dtype=ml_dtypes.bfloat16)
    sel2 = np.zeros((2, 128), dtype=np.float32)
    sel2[0, 0:64] = 1.0
    sel2[1, 64:128] = 1.0
    # tail constants: K=1 denominator-broadcast stationaries (row 64) and the
    # h1 lane-shift identity (rows k -> out partition k+64)
    selr = np.zeros((128, 256), dtype=np.float32)
    selr[:, 0:64] = 1.0
    selr[:, 192:256] = 1.0
    id64 = np.zeros((HD, 128), dtype=np.float32)
    id64[np.arange(HD), np.arange(HD) + 64] = 1.0

    perm = np.concatenate([np.arange(0, HD, 2), np.arange(1, HD, 2)])

    def fold(w):  # [128(m), 1024(d)] -> lhsT layout [128(p), 8r*128+m]
        return np.ascontiguousarray(
            w.reshape(128, NR, 128).transpose(2, 1, 0).reshape(128, DIM)
        ).astype(ml_dtypes.bfloat16)

    in_maps = []
    for c in range(NCORES):
        g = c // 2
        wq_c = wq[128 * c:128 * (c + 1)].reshape(2, HD, DIM)[:, perm, :].reshape(128, DIM)
        wk_g = wk[HD * g:HD * (g + 1)][perm]
        wv_g = wv[HD * g:HD * (g + 1)]
        wkv_c = np.concatenate([wv_g, wk_g], axis=0)        # v rows 0:64, k rows 64:128
        wo_c = np.ascontiguousarray(wo[:, 128 * c:128 * (c + 1)].T).astype(
            ml_dtypes.bfloat16)                              # [128(j), 1024(o)]
        in_maps.append({
            "xT": x_pre,
            "wq_l": fold(wq_c),
            "wkv_l": fold(wkv_c),
            "wo_l": wo_c,
            "trig": trig,
            "tri01": tri01,
            "ident": ident,
            "id64": id64,
            "selr": selr,
            "sel2": sel2,
        })
    return in_maps


def kernel(x, freqs_cos, freqs_sin, wq, wk, wv, wo, _trace=False, _trace_kwargs=None):
    nc = _build()
    in_maps = _host_inputs(x, freqs_cos, freqs_sin, wq, wk, wv, wo)
    kw = {}
    if _trace:
        kw.update(trace=True, **(_trace_kwargs or {}))
    res = run_bass_kernel_spmd(nc, in_maps, core_ids=list(range(NCORES)), **kw)
    acc = np.zeros((128, NQ, NR, SQ), np.float32)
    for c in range(NCORES):
        acc += np.asarray(res.results[c]["out"], np.float32)
    # out[p, n, m, sq] -> [512n+sq, 128m+p]
    out = np.ascontiguousarray(acc.transpose(1, 3, 2, 0)).reshape(1, SEQ, DIM)
    if _trace:
        kernel._last_results = res
    return out

